# revision 25
# baseline (speedup 1.0000x reference)
"""Depth2Normal Trainium2 kernel.

Computes, per batch image: camera-space points from depth + intrinsics, and
per-pixel surface normals via a 9x9-window least-squares plane fit
(AtA n = Atb solved with Cramer's rule; singular windows fall back to Atb),
matching reference.py semantics.

Sharding: pure data parallel, one batch element per NeuronCore (B=8 = 8 cores).

Per-core layout: partitions = H rows (4 tiles of 120 output rows + 4 halo on
each side = 128), free dim = W (640 + 4 zero-pad each side = 648).

Box sums: both window directions run on the TensorEngine. For each output
W-chunk, 9 accumulating matmuls (one per W-shift s in 0..8) with a banded
[128,120] ones matrix as the stationary operand compute the full 2D 9x9 sum
into PSUM. ScalarE evacuates PSUM->SBUF; the per-pixel 3x3 solve runs on the
VectorEngine.
"""

import sys

sys.path.insert(0, "/opt/trn_rl_repo")

import numpy as np

from concourse import bacc, bass, mybir, tile
from concourse.bass_utils import run_bass_kernel_spmd

F32 = mybir.dt.float32
AF = mybir.ActivationFunctionType
OP = mybir.AluOpType

B, H, W = 8, 480, 640
K_SIZE = 9
HALO = K_SIZE // 2  # 4
TILE_H = 120  # output rows per H-tile
N_TILES = H // TILE_H  # 4
PW = W + 2 * HALO  # padded width 648
CW = 320  # W-chunk width (PSUM bank holds 512 f32; 320*4B=1280B)
N_CHUNKS = W // CW  # 2
DEPTH_MAX = 10.0
DET_EPS = 1e-5
NORM_EPS = 1e-5


def _build_program(n_cores=8):
    nc = bacc.Bacc(
        "TRN2",
        target_bir_lowering=False,
        debug=False,
        enable_asserts=False,
        num_devices=n_cores,
    )
    d_depth = nc.dram_tensor("depth", [H, W], F32, kind="ExternalInput").ap()
    # consts layout: [0:120]=band, [120:132]=ysc, [132:135]=kx, [135:135+PW]=xgrid
    d_const = nc.dram_tensor(
        "consts", [128, TILE_H + 3 * N_TILES + 3 + PW], F32, kind="ExternalInput"
    ).ap()
    d_norm = nc.dram_tensor("normals", [3, H, W], F32, kind="ExternalOutput").ap()
    d_pts = nc.dram_tensor("points", [3, H, W], F32, kind="ExternalOutput").ap()

    with tile.TileContext(nc) as tc:
        _body(tc, d_depth, d_const, d_norm, d_pts)
    nc.compile()
    return nc


def _chap(base, slot, ch0, chstep, nch, cw, col0=0):
    """Multi-channel strided view into a [P, n*slot] tile: channels
    ch0, ch0+chstep, ... each [cw] wide starting at col0."""
    v = base[:, ch0 * slot + col0 : ch0 * slot + col0 + cw].copy()
    v.ap = mybir.VecI64Pair([list(v.ap[0]), [chstep * slot, nch], [1, cw]])
    return v


def _body(tc, d_depth, d_const, d_norm, d_pts):
    nc = tc.nc
    from contextlib import ExitStack

    TT = nc.vector.tensor_tensor
    pt_out = lambda p: p[:]
    PWS = PW  # 648 pointwise slot width
    NW = CW + K_SIZE - 1  # 328 box-window input cols per chunk
    SLOT = NW  # solve-tile slot width

    with ExitStack() as ctx:
        const = ctx.enter_context(tc.tile_pool(name="const", bufs=1))
        io = ctx.enter_context(tc.tile_pool(name="io", bufs=1))
        chp = ctx.enter_context(tc.tile_pool(name="ch", bufs=2))
        ps = ctx.enter_context(tc.tile_pool(name="ps", bufs=8, space="PSUM"))
        sv = ctx.enter_context(tc.tile_pool(name="sv", bufs=1))
        sv2 = ctx.enter_context(tc.tile_pool(name="sv2", bufs=2))

        # --- constants (one DMA; slice views) ---
        NCOLS = TILE_H + 3 * N_TILES + 3 + PW
        ct = const.tile([128, NCOLS], F32)
        nc.sync.dma_start(ct[:], d_const[:])
        band = ct[:, 0:TILE_H]
        ysc = ct[:, TILE_H : TILE_H + 3 * N_TILES]
        kx = ct[:, TILE_H + 3 * N_TILES : TILE_H + 3 * N_TILES + 3]
        xgb = ct[:, TILE_H + 3 * N_TILES + 3 : NCOLS]

        # persistent shifted-window operand for the box scan: PZ holds the 9
        # H-summed channels back to back after 9 zero cols; one scan per tile
        # computes all channels' running differences (the first 8 cols of each
        # channel are cross-channel garbage and are never read).
        NPZ = K_SIZE + 9 * PW
        pz = const.tile([TILE_H, NPZ], F32, tag="pz", name="pz")
        nc.gpsimd.memset(pz[:, 0:K_SIZE], 0.0)

        for t in range(N_TILES):
            # --- DD: slot0 = depth (zero-padded), slot1 = dmask ---
            dd = io.tile([128, 2 * PWS], F32, tag="dd", name="dd")
            nc.gpsimd.memset(dd[:, 0:PWS], 0.0)
            rows0 = t * TILE_H - HALO
            r_lo = max(rows0, 0)
            r_hi = min(rows0 + 128, H)
            p_lo = r_lo - rows0
            nc.sync.dma_start(
                dd[p_lo : p_lo + (r_hi - r_lo), HALO : HALO + W], d_depth[r_lo:r_hi, :]
            )
            dep = dd[:, 0:PWS]
            # dmask = depth * (depth<10) * (depth>0), two fused stt ops
            td = io.tile([128, PWS], F32, tag="td", name="td")
            nc.vector.scalar_tensor_tensor(
                td[:], dep, DEPTH_MAX, dep, op0=OP.is_lt, op1=OP.mult
            )
            nc.vector.scalar_tensor_tensor(
                dd[:, PWS : 2 * PWS], dep, 0.0, td[:], op0=OP.is_gt, op1=OP.mult
            )

            # --- P_i then (mp_i, cam_i) dual products ---
            # CH slots: 0-5 = outer products s00..s22, 6-8 = mp, 9-11 = cam
            p3 = io.tile([128, 3 * PWS], F32, tag="p3", name="p3")
            ch = chp.tile([128, 12 * PWS], F32, tag="ch", name="ch", bufs=1)
            for i in range(3):
                nc.scalar.activation(
                    p3[:, i * PWS : (i + 1) * PWS],
                    xgb,
                    AF.Identity,
                    bias=ysc[:, 3 * t + i : 3 * t + i + 1],
                    scale=kx[:, i : i + 1],
                )
                # out (mp_i@6+i, cam_i@9+i), in1 = (dmask@1, depth@0)
                TT(
                    _chap(ch, PWS, 6 + i, 3, 2, PWS),
                    _chap(p3, PWS, i, 0, 2, PWS),
                    _chap(dd, PWS, 1, -1, 2, PWS),
                    OP.mult,
                )
                nc.sync.dma_start(
                    d_pts[i, t * TILE_H : (t + 1) * TILE_H, :],
                    ch[HALO : HALO + TILE_H, (9 + i) * PWS + HALO : (9 + i) * PWS + HALO + W],
                )
            # outer products: diag via ACT Square, off-diag via dual + single
            for i, s in ((0, 0), (1, 3), (2, 5)):
                nc.scalar.square(
                    ch[:, s * PWS : s * PWS + PWS], ch[:, (6 + i) * PWS : (7 + i) * PWS]
                )
            TT(
                _chap(ch, PWS, 1, 1, 2, PWS),
                _chap(ch, PWS, 6, 0, 2, PWS),
                _chap(ch, PWS, 7, 1, 2, PWS),
                OP.mult,
            )
            TT(
                ch[:, 4 * PWS : 5 * PWS],
                ch[:, 7 * PWS : 8 * PWS],
                ch[:, 8 * PWS : 9 * PWS],
                OP.mult,
            )

            # --- full-width box sums + solve (one pass, cw=640) ---
            # H-band matmuls (N<=512 forces a 512+136 split) -> PSUM; ScalarE
            # assembles PZ = [9 zeros | P(648)] in SBUF; one DVE scan per
            # channel computes the 9-wide running difference (2D box sum).
            WIN = W + K_SIZE - 1  # 648 window input cols
            SB = WIN  # BX slot
            S2W = W  # solve slot
            bx = sv2.tile([TILE_H, 9 * SB], F32, tag="bx", name="bx", bufs=1)
            for ci in range(9):
                pa = ps.tile([TILE_H, 512], F32, tag="pa", name="pa", bufs=4)
                nc.tensor.matmul(
                    pt_out(pa), band, ch[:, ci * PWS : ci * PWS + 512],
                    start=True, stop=True,
                )
                pb = ps.tile([TILE_H, WIN - 512], F32, tag="pb", name="pb", bufs=4)
                nc.tensor.matmul(
                    pt_out(pb), band, ch[:, ci * PWS + 512 : ci * PWS + WIN],
                    start=True, stop=True,
                )
                nc.scalar.copy(
                    pz[:, K_SIZE + ci * PWS : K_SIZE + ci * PWS + 512], pa[:]
                )
                nc.scalar.copy(
                    pz[:, K_SIZE + ci * PWS + 512 : K_SIZE + (ci + 1) * PWS], pb[:]
                )
            nc.vector.tensor_tensor_scan(
                bx[:],
                pz[:, K_SIZE:NPZ],
                pz[:, 0 : NPZ - K_SIZE],
                0.0,
                OP.add,
                OP.subtract,
            )
            BOX0 = K_SIZE - 1  # box[j] lives at bx col j+8
            B = lambda ch0, chstep, nch: _chap(bx, SB, ch0, chstep, nch, S2W, col0=BOX0)
            # BX channels: s00=0 s01=1 s02=2 s11=3 s12=4 s22=5 b0=6 b1=7 b2=8

            # --- cofactor products PR: PA=0-5, PB=6-11; M = PA - PB ---
            pr = sv.tile([TILE_H, 12 * S2W], F32, tag="work12", name="pr")
            P = lambda ch0, chstep, nch: _chap(pr, S2W, ch0, chstep, nch, S2W)
            TT(P(3, 6, 2), B(1, 1, 2), B(5, -1, 2), OP.mult)   # s01*s22, s02*s12
            TT(P(5, 6, 2), B(1, 1, 2), B(4, -1, 2), OP.mult)   # s01*s12, s02*s11
            TT(P(4, 6, 2), B(1, 1, 2), B(8, -1, 2), OP.mult)   # s01*b2,  s02*b1
            TT(P(1, 6, 2), B(7, -3, 2), B(5, 3, 2), OP.mult)   # b1*s22,  s12*b2
            TT(P(0, 8, 2), B(3, 0, 2), B(5, 3, 2), OP.mult)    # s11*s22, s11*b2
            TT(P(2, 1, 1), B(7, 1, 1), B(4, 1, 1), OP.mult)    # b1*s12
            nc.scalar.activation(
                pr[:, 6 * S2W : 7 * S2W],
                bx[:, 4 * SB + BOX0 : 4 * SB + BOX0 + S2W],
                AF.Square,
            )  # s12^2
            m6 = sv.tile([TILE_H, 6 * S2W], F32, tag="m6", name="m6")
            M = lambda ch0, chstep, nch: _chap(m6, S2W, ch0, chstep, nch, S2W)
            TT(M(0, 1, 6), P(0, 1, 6), P(6, 1, 6), OP.subtract)
            # M slots: M1=0 M2=1 M3=2 M4=3 M5=4 M6=5

            # --- det/d products DP (same slot as PR, freed by the M-sub):
            # A=[s00M1,b0M1,s00M2,b0M6] 0-3, B=[s01M4,s01M2,b0M4,s00M3] 4-7,
            # C=[s02M6,s02M3,s02M5,s01M5] 8-11
            dp = sv.tile([TILE_H, 12 * S2W], F32, tag="work12", name="dp")
            D = lambda ch0, chstep, nch: _chap(dp, S2W, ch0, chstep, nch, S2W)
            TT(D(0, 2, 2), B(0, 0, 2), M(0, 1, 2), OP.mult)    # s00M1, s00M2
            TT(D(7, 1, 1), B(0, 1, 1), M(2, 1, 1), OP.mult)    # s00M3
            TT(D(4, 1, 2), B(1, 0, 2), M(3, -2, 2), OP.mult)   # s01M4, s01M2
            TT(D(11, 1, 1), B(1, 1, 1), M(4, 1, 1), OP.mult)   # s01M5
            TT(D(8, 1, 2), B(2, 0, 2), M(5, -3, 2), OP.mult)   # s02M6, s02M3
            TT(D(10, 1, 1), B(2, 1, 1), M(4, 1, 1), OP.mult)   # s02M5
            TT(D(1, 5, 2), B(6, 0, 2), M(0, 3, 2), OP.mult)    # b0M1, b0M4
            TT(D(3, 1, 1), B(6, 1, 1), M(5, 1, 1), OP.mult)    # b0M6
            st = sv.tile([TILE_H, 4 * S2W], F32, tag="st", name="st")
            S = lambda ch0, chstep, nch: _chap(st, S2W, ch0, chstep, nch, S2W)
            TT(S(0, 1, 4), D(0, 1, 4), D(4, 1, 4), OP.subtract)
            s2 = sv2.tile([TILE_H, 4 * S2W], F32, tag="s2", name="s2", bufs=1)
            S2 = lambda ch0, chstep, nch: _chap(s2, S2W, ch0, chstep, nch, S2W)
            TT(S2(0, 1, 3), S(0, 1, 3), D(8, 1, 3), OP.add)
            TT(S2(3, 1, 1), S(3, 1, 1), D(11, 1, 1), OP.subtract)

            # --- good = det >= eps; blend m = good ? d : Atb; normalize ---
            good = sv2.tile([TILE_H, S2W], mybir.dt.uint8, tag="good", name="good")
            nc.vector.tensor_scalar(good[:], s2[:, 0:S2W], DET_EPS, None, op0=OP.is_ge)
            m3 = sv.tile([TILE_H, 3 * S2W], F32, tag="st", name="m3")
            for i in range(3):
                nc.scalar.copy(
                    m3[:, i * S2W : (i + 1) * S2W],
                    bx[:, (6 + i) * SB + BOX0 : (6 + i) * SB + BOX0 + S2W],
                )
            for i in range(3):
                nc.vector.copy_predicated(
                    m3[:, i * S2W : (i + 1) * S2W], good[:], s2[:, (1 + i) * S2W : (2 + i) * S2W]
                )
            qt = sv.tile([TILE_H, 3 * S2W], F32, tag="m6", name="qt")
            for i in range(3):
                nc.scalar.square(
                    qt[:, i * S2W : (i + 1) * S2W], m3[:, i * S2W : (i + 1) * S2W]
                )
            q = sv.tile([TILE_H, S2W], F32, tag="q", name="q")
            TT(q[:], qt[:, 0:S2W], qt[:, S2W : 2 * S2W], OP.add)
            q2 = sv.tile([TILE_H, S2W], F32, tag="q2", name="q2")
            TT(q2[:], q[:], qt[:, 2 * S2W : 3 * S2W], OP.add)
            sn = sv.tile([TILE_H, S2W], F32, tag="sn", name="sn")
            nc.scalar.sqrt(sn[:], q2[:])
            sn2 = sv.tile([TILE_H, S2W], F32, tag="sn2", name="sn2")
            nc.vector.tensor_scalar_add(sn2[:], sn[:], NORM_EPS)
            rn = sv.tile([TILE_H, S2W], F32, tag="rn", name="rn")
            nc.vector.reciprocal_approx_fast(rn[:], sn2[:])
            o3 = sv2.tile([TILE_H, 3 * S2W], F32, tag="o3", name="o3", bufs=1)
            rv = rn[:, 0:S2W].copy()
            rv.ap = mybir.VecI64Pair([list(rv.ap[0]), [0, 3], [1, S2W]])
            TT(_chap(o3, S2W, 0, 1, 3, S2W), _chap(m3, S2W, 0, 1, 3, S2W), rv, OP.mult)
            for i in range(3):
                nc.sync.dma_start(
                    d_norm[i, t * TILE_H : (t + 1) * TILE_H, :],
                    o3[:, i * S2W : (i + 1) * S2W],
                )


def _host_constants(Kinv):
    """Per-core constant tensors derived from one [3,3] inverse intrinsic."""
    Ki = Kinv.astype(np.float32)
    band = np.zeros((128, TILE_H), np.float32)
    for k in range(128):
        lo, hi = max(0, k - 2 * HALO), min(TILE_H - 1, k)
        if lo <= hi:
            band[k, lo : hi + 1] = 1.0
    xg = np.tile((np.arange(PW, dtype=np.float32) - HALO)[None, :], (128, 1))
    ysc = np.zeros((128, 3 * N_TILES), np.float32)
    p = np.arange(128, dtype=np.float32)
    for t in range(N_TILES):
        hrow = t * TILE_H - HALO + p
        for i in range(3):
            ysc[:, 3 * t + i] = Ki[i, 1] * hrow + Ki[i, 2]
    kx = np.tile(Ki[:, 0][None, :], (128, 1)).astype(np.float32)
    return {"consts": np.concatenate([band, ysc, kx, xg], axis=1)}


_PROGRAM = None


def _program():
    global _PROGRAM
    if _PROGRAM is None:
        _PROGRAM = _build_program()
    return _PROGRAM


def _run(depth, intrinsic_inv, trace=False, **kw):
    nc = _program()
    in_maps = []
    for b in range(B):
        m = {"depth": np.ascontiguousarray(depth[b], dtype=np.float32)}
        m.update(_host_constants(np.asarray(intrinsic_inv[b])))
        in_maps.append(m)
    return run_bass_kernel_spmd(nc, in_maps, core_ids=list(range(B)), trace=trace, **kw)


def kernel(depth, intrinsic_inv):
    res = _run(np.asarray(depth), np.asarray(intrinsic_inv), trace=False)
    normals = np.stack([res.results[b]["normals"] for b in range(B)])
    points = np.stack([res.results[b]["points"] for b in range(B)])
    return normals.astype(np.float32), points.astype(np.float32)
